# revision 41
# baseline (speedup 1.0000x reference)
"""Gated TCN layer (fully conditioned) as a Bass/Tile kernel on 8 NeuronCores.

Reference computation (per sample b):
    kern = (c @ adapter_w + adapter_b).reshape(2*CH, CH, K)
    y    = dilated causal conv of x with per-sample kern (K=3, dil=4)
    y   += (c @ bias_w + bias_b)[:, None]
    z    = tanh(y[:CH]) * sigmoid(y[CH:])
    out  = resi_w @ z + resi_b + x
Returns (out, z).

Sharding: data-parallel over batch, 2 samples per core.  The two samples
are packed on the 128 SBUF partitions (sample 0 on 0-63, sample 1 on
64-127) so conv / activations / residual all run at full 128-partition
width.  Conv weights are block-diagonal per tap: T_k pairs the tanh
halves of both samples, S_k the sigmoid halves, giving 128-row PE
contraction per matmul.  The residual rw2 @ z2 runs software-pipelined
one block behind the conv so the PE never waits on the gate output;
(+ rb + x) is a single fused DVE op.
"""

import numpy as np

from concourse import bacc, mybir, tile
from concourse.bass_utils import run_bass_kernel_spmd

K = 3
DIL = 4
CH = 64
COND = 128
B, T = 16, 16384
NCORES = 8
BL = B // NCORES          # samples per core
PAD = (K - 1) * DIL       # causal left pad = 8
NT = 512                  # matmul free-dim (one PSUM bank of fp32)
UW = 1024                 # processing unit width (2 PSUM banks)
NJ = T // UW
F = K * CH * 2 * CH       # 24576 adapter columns
FI = 2 * CH * K           # 384 adapter columns per input-channel row
XCH = 2048                # x load chunk (columns)
P = BL * CH               # 128 partitions = both samples' channels

F32 = mybir.dt.float32
F32R = mybir.dt.float32r
BF16 = mybir.dt.bfloat16
AF = mybir.ActivationFunctionType

ADAPTER_BF16 = True

# Set by test.py to capture a profile; harness path leaves these alone.
TRACE = False
LAST_RESULTS = None

_NC = None


def _build():
    nc = bacc.Bacc("TRN2", target_bir_lowering=False, debug=False)

    x_in = nc.dram_tensor("x_in", [P, T], F32R, kind="ExternalInput")
    cT_d = nc.dram_tensor("cT", [COND, BL], BF16 if ADAPTER_BF16 else F32R, kind="ExternalInput")
    cTf_d = nc.dram_tensor("cTf", [COND, BL], F32R, kind="ExternalInput")
    aw_d = nc.dram_tensor("aw_r", [COND, F], BF16 if ADAPTER_BF16 else F32R, kind="ExternalInput")
    ab_d = nc.dram_tensor("ab_r", [P, FI], F32R, kind="ExternalInput")
    bw_d = nc.dram_tensor("bw", [COND, 2 * CH], F32R, kind="ExternalInput")
    bb_d = nc.dram_tensor("bb", [1, 2 * CH], F32R, kind="ExternalInput")
    rw2_d = nc.dram_tensor("rw2", [P, P], F32R, kind="ExternalInput")
    rb2_d = nc.dram_tensor("rb2", [P, 1], F32, kind="ExternalInput")
    out_d = nc.dram_tensor("out_d", [P, T], F32, kind="ExternalOutput")
    z_d = nc.dram_tensor("z_d", [P, T], F32R, kind="ExternalOutput")  # f32r == f32 bits

    with tile.TileContext(nc) as tc:
        with (
            tc.tile_pool(name="const", bufs=1) as constp,
            tc.tile_pool(name="xpool", bufs=1) as xpool,
            tc.tile_pool(name="kern", bufs=1) as kernp,
        ):
            # cT first (the only input the adapter matmuls need beyond aw);
            # every descriptor ahead of the aw chunks delays them by the
            # ~620ns dma_start issue cost, so nothing else goes first.
            cT_sb = constp.tile([COND, BL], BF16 if ADAPTER_BF16 else F32R)
            nc.sync.dma_start(cT_sb[:, :], cT_d[:, :])
            cTf_sb = constp.tile([COND, BL], F32R)
            bw_sb = constp.tile([COND, 2 * CH], F32R)
            bb_sb = constp.tile([1, 2 * CH], F32R)
            ones_sb = constp.tile([1, BL], F32R)
            nc.vector.memset(ones_sb[:, :].bitcast(F32), 1.0)

            xbuf = xpool.tile([P, PAD + T], F32R)
            nc.vector.memset(xbuf[:, 0:PAD].bitcast(F32), 0.0)

            ab_sb = constp.tile([P, FI], F32R)
            rw2_sb = constp.tile([P, P], F32R)
            rb2_sb = constp.tile([P, 1], F32)

            # Per-sample dynamic kernels: kfin[64b+i, 128k+o] = kern_b[o,i,k]+ab.
            # KT holds the block-diagonal paired weights:
            #   cols [128k,128k+128)      = T_k (tanh pair of tap k)
            #   cols [384+128k, ...)      = S_k (sigmoid pair of tap k)
            kern_raw = kernp.tile([P, FI], F32R, name="kern_raw")
            kt = kernp.tile([P, 2 * FI], F32R, name="kt")
            nc.vector.memset(kt[:, :].bitcast(F32), 0.0)
            bias_sb = constp.tile([2 * CH, BL], F32)
            bt2 = constp.tile([P, 1], F32)   # paired tanh bias
            bs2 = constp.tile([P, 1], F32)   # paired sigmoid bias

            # ---------------- phase A: adapter + conditioned bias ----------
            with (
                tc.tile_pool(name="awp", bufs=4) as awp,
                tc.tile_pool(name="apsum", bufs=7, space="PSUM") as apsum,
                tc.tile_pool(name="stg", bufs=3) as stgp,
                tc.tile_pool(name="bpsum", bufs=1, space="PSUM") as bpsum,
            ):
                # all adapter-weight chunks issued up front on the sync queue
                # (6KB per-partition lines: the DMA engines' sweet spot)
                awts = []
                for c in range(8):
                    awt = awp.tile([COND, 8 * FI], BF16 if ADAPTER_BF16 else F32R, tag="aw")
                    nc.sync.dma_start(
                        awt[:, :], aw_d[:, c * 8 * FI : (c + 1) * 8 * FI]
                    )
                    awts.append(awt)
                # bias inputs ride the idle gpsimd queue so the bias
                # matmul (and the paired-bias DMAs below) finish early
                nc.gpsimd.dma_start(cTf_sb[:, :], cTf_d[:, :])
                nc.gpsimd.dma_start(bw_sb[:, :], bw_d[:, :])
                nc.gpsimd.dma_start(bb_sb[:, :], bb_d[:, :])
                pb = bpsum.tile([2 * CH, BL], F32)
                nc.tensor.matmul(
                    pb[:, :], bw_sb[:, :], cTf_sb[:, :], start=True, stop=False
                )
                nc.tensor.matmul(
                    pb[:, :], bb_sb[:, :], ones_sb[:, :], start=False, stop=True
                )
                nc.vector.tensor_copy(bias_sb[:, :], pb[:, :])
                # paired per-partition bias tiles: issued now (the sync
                # engine only waits ~1us for the pb copy here), keeping these
                # 4 descriptors out of the critical post-adapter window
                for b in range(BL):
                    nc.sync.dma_start(
                        bt2[CH * b : CH * (b + 1), :], bias_sb[0:CH, b : b + 1]
                    )
                    nc.sync.dma_start(
                        bs2[CH * b : CH * (b + 1), :], bias_sb[CH : 2 * CH, b : b + 1]
                    )

                # 16 groups of 4 input-channel rows, staged in quads of
                # [4,4,4,2,2] groups: few scatters (dma_start issue costs
                # ~620ns), with small final quads so the last scatters'
                # single-partition-line transfer latency doesn't delay the
                # kernel assembly.
                QUADS = [4, 4, 4, 2, 2]
                QEND = []
                acc = 0
                for ng in QUADS:
                    acc += ng
                    QEND.append(acc - 1)
                qi = 0
                qstart = 0
                stg = None
                for g in range(CH // 4):
                    if g % 2 == 0:
                        awt = awts[g // 2]
                        aoff = g * 4 * FI
                    if g == qstart:
                        ng = QUADS[qi]
                        stg = stgp.tile([BL, 4 * ng * FI], F32R, tag=f"stg{ng}")
                    so = (g - qstart) * 4 * FI
                    for u in range(4):
                        j = 4 * g + u
                        ps = apsum.tile([BL, FI], F32, tag="ap")
                        nc.tensor.matmul(
                            ps[:, :],
                            cT_sb[:, :],
                            awt[:, j * FI - aoff : (j + 1) * FI - aoff],
                            start=True,
                            stop=True,
                        )
                        if u % 2 == 0:
                            nc.scalar.activation(
                                stg[:, so + u * FI : so + (u + 1) * FI], ps[:, :], AF.Copy
                            )
                        else:
                            nc.vector.tensor_copy(
                                stg[:, so + u * FI : so + (u + 1) * FI], ps[:, :]
                            )
                    if g == QEND[qi]:
                        # the two samples' scatters go to different queues so
                        # their single-partition-line transfers run on two
                        # DMA engines concurrently instead of serializing
                        for b in range(BL):
                            eng = nc.sync if b == 0 else nc.gpsimd
                            eng.dma_start(
                                kern_raw[CH * b + 4 * qstart : CH * b + 4 * (g + 1), :],
                                stg[b : b + 1, :],
                            )
                        qstart = g + 1
                        qi += 1
                    if g == 6:
                        nc.sync.dma_start(ab_sb[:, :], ab_d[:, :])
                        nc.sync.dma_start(rw2_sb[:, :], rw2_d[:, :])
                        nc.sync.dma_start(rb2_sb[:, :], rb2_d[:, :])
                # x chunks queue behind the scatters on the sync queue, so
                # the bulk x flood can't delay the kernel assembly.
                XCHUNKS = [1024] + [2048] * 7 + [1024]
                xq = 0
                for xl in XCHUNKS:
                    nc.sync.dma_start(
                        xbuf[:, PAD + xq : PAD + xq + xl],
                        x_in[:, xq : xq + xl],
                    )
                    xq += xl
                # block-diagonal paired tiles built directly from kern_raw
                # + host-prelayouted adapter bias (ab2kt is already in KT
                # coordinates, zeros in the off-diagonal blocks); T blocks
                # first so the conv's first matmuls unblock earliest
                for k in range(K):
                    nc.vector.tensor_add(
                        kt[0:CH, 128 * k : 128 * k + CH],
                        kern_raw[0:CH, 128 * k : 128 * k + CH],
                        ab_sb[0:CH, 128 * k : 128 * k + CH],
                    )
                    nc.vector.tensor_add(
                        kt[CH:P, 128 * k + CH : 128 * (k + 1)],
                        kern_raw[CH:P, 128 * k : 128 * k + CH],
                        ab_sb[CH:P, 128 * k : 128 * k + CH],
                    )
                for k in range(K):
                    nc.vector.tensor_add(
                        kt[0:CH, FI + 128 * k : FI + 128 * k + CH],
                        kern_raw[0:CH, 128 * k + CH : 128 * (k + 1)],
                        ab_sb[0:CH, 128 * k + CH : 128 * (k + 1)],
                    )
                    nc.vector.tensor_add(
                        kt[CH:P, FI + 128 * k + CH : FI + 128 * (k + 1)],
                        kern_raw[CH:P, 128 * k + CH : 128 * (k + 1)],
                        ab_sb[CH:P, 128 * k + CH : 128 * (k + 1)],
                    )

            # ---------------- phase B: conv + gate + residual --------------
            # The residual matmul for block j-1 is emitted after block j's
            # conv matmuls: by then z2[j-1] is long finished, so the PE
            # stream never stalls on the gate.  (+rb +x) is one fused DVE op.
            with (
                tc.tile_pool(name="ppool", bufs=2, space="PSUM") as ppool,
                tc.tile_pool(name="spool", bufs=2, space="PSUM") as spool,
                tc.tile_pool(name="work", bufs=2) as workp,
                tc.tile_pool(name="store", bufs=4) as storep,
            ):
                z2s = [None] * NJ
                po2s = [None] * NJ

                def emit_residual(jr):
                    po2 = ppool.tile([P, UW], F32, tag="acc")
                    po2s[jr] = po2
                    for h in range(UW // NT):
                        nc.tensor.matmul(
                            po2[:, h * NT : (h + 1) * NT],
                            rw2_sb[:, :],
                            z2s[jr][:, h * NT : (h + 1) * NT],
                            start=True,
                            stop=True,
                        )

                def emit_post(jr):
                    c0r = jr * UW
                    ot = storep.tile([P, UW], F32, tag="ot")
                    nc.vector.scalar_tensor_tensor(
                        ot[:, :],
                        po2s[jr][:, :],
                        rb2_sb[:, 0:1],
                        xbuf[:, c0r + PAD : c0r + PAD + UW],
                        op0=mybir.AluOpType.add,
                        op1=mybir.AluOpType.add,
                    )
                    nc.gpsimd.dma_start(out_d[:, c0r : c0r + UW], ot[:, :])

                for j in range(NJ):
                    c0 = j * UW
                    py_t = ppool.tile([P, UW], F32, tag="acc")
                    py_s = spool.tile([P, UW], F32, tag="pys")
                    for k in range(K):
                        for h in range(UW // NT):
                            nc.tensor.matmul(
                                py_t[:, h * NT : (h + 1) * NT],
                                kt[:, 128 * k : 128 * (k + 1)],
                                xbuf[:, c0 + h * NT + DIL * k : c0 + h * NT + DIL * k + NT],
                                start=(k == 0),
                                stop=(k == K - 1),
                            )
                    for k in range(K):
                        for h in range(UW // NT):
                            nc.tensor.matmul(
                                py_s[:, h * NT : (h + 1) * NT],
                                kt[:, FI + 128 * k : FI + 128 * (k + 1)],
                                xbuf[:, c0 + h * NT + DIL * k : c0 + h * NT + DIL * k + NT],
                                start=(k == 0),
                                stop=(k == K - 1),
                            )
                    if j > 0:
                        emit_residual(j - 1)
                    ta = workp.tile([P, UW], F32R, tag="ta")
                    nc.scalar.activation(ta[:, :], py_t[:, :], AF.Tanh, bias=bt2[:, 0:1])
                    ts = workp.tile([P, UW], F32R, tag="ts")
                    nc.scalar.activation(ts[:, :], py_s[:, :], AF.Sigmoid, bias=bs2[:, 0:1])
                    z2 = storep.tile([P, UW], F32R, tag="z2")
                    z2s[j] = z2
                    nc.vector.tensor_mul(z2[:, :], ta[:, :], ts[:, :])
                    nc.gpsimd.dma_start(z_d[:, c0 : c0 + UW], z2[:, :])
                    if j > 0:
                        emit_post(j - 1)
                emit_residual(NJ - 1)
                emit_post(NJ - 1)

    nc.compile()
    return nc


def get_nc():
    global _NC
    if _NC is None:
        _NC = _build()
    return _NC


def make_in_maps(inputs):
    x = np.ascontiguousarray(np.asarray(inputs["x"], np.float32))
    c = np.asarray(inputs["c"], np.float32)
    aw = np.asarray(inputs["adapter_w"], np.float32)
    ab = np.asarray(inputs["adapter_b"], np.float32)
    bw = np.ascontiguousarray(np.asarray(inputs["bias_w"], np.float32))
    bb = np.asarray(inputs["bias_b"], np.float32).reshape(1, 2 * CH)
    rw = np.asarray(inputs["resi_w"], np.float32)
    rb = np.asarray(inputs["resi_b"], np.float32)

    # adapter columns [cond, (o,i,k)] -> [cond, (i,k,o)]
    aw_r = np.ascontiguousarray(
        aw.reshape(COND, 2 * CH, CH, K).transpose(0, 2, 3, 1).reshape(COND, F)
    )
    ab_r1 = ab.reshape(2 * CH, CH, K).transpose(1, 2, 0).reshape(CH, FI)
    ab_r = np.ascontiguousarray(np.concatenate([ab_r1, ab_r1], axis=0))
    # paired residual weights: blockdiag(rw.T, rw.T), [rb; rb]
    rw2 = np.zeros((P, P), np.float32)
    rw2[0:CH, 0:CH] = rw.T
    rw2[CH:P, CH:P] = rw.T
    rb2 = np.concatenate([rb, rb]).reshape(P, 1).astype(np.float32)

    import ml_dtypes

    adt = ml_dtypes.bfloat16 if ADAPTER_BF16 else np.float32
    aw_bf = np.ascontiguousarray(aw_r.astype(adt))
    in_maps = []
    for m in range(NCORES):
        sl = slice(BL * m, BL * (m + 1))
        in_maps.append(
            {
                "x_in": np.ascontiguousarray(x[sl].reshape(P, T)),
                "cT": np.ascontiguousarray(c[sl].T.astype(adt)),
                "cTf": np.ascontiguousarray(c[sl].T),
                "aw_r": aw_bf,
                "ab_r": ab_r,
                "bw": bw,
                "bb": bb,
                "rw2": rw2,
                "rb2": rb2,
            }
        )
    return in_maps


def kernel(**inputs):
    global LAST_RESULTS
    nc = get_nc()
    in_maps = make_in_maps(inputs)
    res = run_bass_kernel_spmd(
        nc, in_maps, list(range(NCORES)), trace=TRACE
    )
    LAST_RESULTS = res
    out = np.empty((B, CH, T), np.float32)
    z = np.empty((B, CH, T), np.float32)
    for m in range(NCORES):
        out[BL * m : BL * (m + 1)] = res.results[m]["out_d"].reshape(BL, CH, T)
        z[BL * m : BL * (m + 1)] = res.results[m]["z_d"].reshape(BL, CH, T)
    return out, z


# revision 42
# speedup vs baseline: 1.0406x; 1.0406x over previous
"""Gated TCN layer (fully conditioned) as a Bass/Tile kernel on 8 NeuronCores.

Reference computation (per sample b):
    kern = (c @ adapter_w + adapter_b).reshape(2*CH, CH, K)
    y    = dilated causal conv of x with per-sample kern (K=3, dil=4)
    y   += (c @ bias_w + bias_b)[:, None]
    z    = tanh(y[:CH]) * sigmoid(y[CH:])
    out  = resi_w @ z + resi_b + x
Returns (out, z).

Sharding: data-parallel over batch, 2 samples per core.  The two samples
are packed on the 128 SBUF partitions (sample 0 on 0-63, sample 1 on
64-127) so conv / activations / residual all run at full 128-partition
width.  Conv weights are block-diagonal per tap: T_k pairs the tanh
halves of both samples, S_k the sigmoid halves, giving 128-row PE
contraction per matmul.  The residual rw2 @ z2 runs software-pipelined
one block behind the conv so the PE never waits on the gate output;
(+ rb + x) is a single fused DVE op.
"""

import numpy as np

from concourse import bacc, mybir, tile
from concourse.bass_utils import run_bass_kernel_spmd

K = 3
DIL = 4
CH = 64
COND = 128
B, T = 16, 16384
NCORES = 8
BL = B // NCORES          # samples per core
PAD = (K - 1) * DIL       # causal left pad = 8
NT = 512                  # matmul free-dim (one PSUM bank of fp32)
UW = 1024                 # processing unit width (2 PSUM banks)
NJ = T // UW
F = K * CH * 2 * CH       # 24576 adapter columns
FI = 2 * CH * K           # 384 adapter columns per input-channel row
XCH = 2048                # x load chunk (columns)
P = BL * CH               # 128 partitions = both samples' channels

F32 = mybir.dt.float32
F32R = mybir.dt.float32r
BF16 = mybir.dt.bfloat16
AF = mybir.ActivationFunctionType

ADAPTER_BF16 = True

# Set by test.py to capture a profile; harness path leaves these alone.
TRACE = False
LAST_RESULTS = None

_NC = None


def _build():
    nc = bacc.Bacc("TRN2", target_bir_lowering=False, debug=False)

    x_in = nc.dram_tensor("x_in", [P, T], F32R, kind="ExternalInput")
    cT_d = nc.dram_tensor("cT", [COND, BL], BF16 if ADAPTER_BF16 else F32R, kind="ExternalInput")
    cTf_d = nc.dram_tensor("cTf", [COND, BL], F32R, kind="ExternalInput")
    aw_d = nc.dram_tensor("aw_r", [COND, F], BF16 if ADAPTER_BF16 else F32R, kind="ExternalInput")
    ab_d = nc.dram_tensor("ab_r", [P, FI], F32R, kind="ExternalInput")
    bw_d = nc.dram_tensor("bw", [COND, 2 * CH], F32R, kind="ExternalInput")
    bb_d = nc.dram_tensor("bb", [1, 2 * CH], F32R, kind="ExternalInput")
    rw2_d = nc.dram_tensor("rw2", [P, P], F32R, kind="ExternalInput")
    rb2_d = nc.dram_tensor("rb2", [P, 1], F32, kind="ExternalInput")
    out_d = nc.dram_tensor("out_d", [P, T], F32, kind="ExternalOutput")
    z_d = nc.dram_tensor("z_d", [P, T], F32R, kind="ExternalOutput")  # f32r == f32 bits

    with tile.TileContext(nc) as tc:
        with (
            tc.tile_pool(name="const", bufs=1) as constp,
            tc.tile_pool(name="xpool", bufs=1) as xpool,
            tc.tile_pool(name="kern", bufs=1) as kernp,
        ):
            # cT first (the only input the adapter matmuls need beyond aw);
            # every descriptor ahead of the aw chunks delays them by the
            # ~620ns dma_start issue cost, so nothing else goes first.
            cT_sb = constp.tile([COND, BL], BF16 if ADAPTER_BF16 else F32R)
            nc.sync.dma_start(cT_sb[:, :], cT_d[:, :])
            cTf_sb = constp.tile([COND, BL], F32R)
            bw_sb = constp.tile([COND, 2 * CH], F32R)
            bb_sb = constp.tile([1, 2 * CH], F32R)
            ones_sb = constp.tile([1, BL], F32R)
            nc.vector.memset(ones_sb[:, :].bitcast(F32), 1.0)

            xbuf = xpool.tile([P, PAD + T], F32R)
            nc.vector.memset(xbuf[:, 0:PAD].bitcast(F32), 0.0)

            ab_sb = constp.tile([P, FI], F32R)
            rw2_sb = constp.tile([P, P], F32R)
            rb2_sb = constp.tile([P, 1], F32)

            # Per-sample dynamic kernels: kfin[64b+i, 128k+o] = kern_b[o,i,k]+ab.
            # KT holds the block-diagonal paired weights:
            #   cols [128k,128k+128)      = T_k (tanh pair of tap k)
            #   cols [384+128k, ...)      = S_k (sigmoid pair of tap k)
            kern_raw = kernp.tile([P, FI], F32R, name="kern_raw")
            kt = kernp.tile([P, 2 * FI], F32R, name="kt")
            nc.vector.memset(kt[:, :].bitcast(F32), 0.0)
            bias_sb = constp.tile([2 * CH, BL], F32)
            bt2 = constp.tile([P, 1], F32)   # paired tanh bias
            bs2 = constp.tile([P, 1], F32)   # paired sigmoid bias

            # ---------------- phase A: adapter + conditioned bias ----------
            with (
                tc.tile_pool(name="awp", bufs=4) as awp,
                tc.tile_pool(name="apsum", bufs=7, space="PSUM") as apsum,
                tc.tile_pool(name="stg", bufs=3) as stgp,
                tc.tile_pool(name="bpsum", bufs=1, space="PSUM") as bpsum,
            ):
                # all adapter-weight chunks issued up front on the sync queue
                # (6KB per-partition lines: the DMA engines' sweet spot)
                # aw chunks split across the sync and gpsimd queues so two
                # queues stream the adapter weights concurrently (the
                # adapter matmul chain is aw-DMA-paced)
                awts = []
                for c in range(8):
                    awt = awp.tile([COND, 8 * FI], BF16 if ADAPTER_BF16 else F32R, tag="aw")
                    eng = nc.sync if c % 2 == 0 else nc.gpsimd
                    eng.dma_start(
                        awt[:, :], aw_d[:, c * 8 * FI : (c + 1) * 8 * FI]
                    )
                    awts.append(awt)
                # bias inputs follow on the gpsimd queue
                nc.gpsimd.dma_start(cTf_sb[:, :], cTf_d[:, :])
                nc.gpsimd.dma_start(bw_sb[:, :], bw_d[:, :])
                nc.gpsimd.dma_start(bb_sb[:, :], bb_d[:, :])
                pb = bpsum.tile([2 * CH, BL], F32)
                nc.tensor.matmul(
                    pb[:, :], bw_sb[:, :], cTf_sb[:, :], start=True, stop=False
                )
                nc.tensor.matmul(
                    pb[:, :], bb_sb[:, :], ones_sb[:, :], start=False, stop=True
                )
                nc.vector.tensor_copy(bias_sb[:, :], pb[:, :])
                # paired per-partition bias tiles: issued now (the sync
                # engine only waits ~1us for the pb copy here), keeping these
                # 4 descriptors out of the critical post-adapter window
                for b in range(BL):
                    nc.sync.dma_start(
                        bt2[CH * b : CH * (b + 1), :], bias_sb[0:CH, b : b + 1]
                    )
                    nc.sync.dma_start(
                        bs2[CH * b : CH * (b + 1), :], bias_sb[CH : 2 * CH, b : b + 1]
                    )

                # 16 groups of 4 input-channel rows, staged in quads of
                # [4,4,4,2,2] groups: few scatters (dma_start issue costs
                # ~620ns), with small final quads so the last scatters'
                # single-partition-line transfer latency doesn't delay the
                # kernel assembly.
                QUADS = [4, 4, 4, 2, 2]
                QEND = []
                acc = 0
                for ng in QUADS:
                    acc += ng
                    QEND.append(acc - 1)
                qi = 0
                qstart = 0
                stg = None
                for g in range(CH // 4):
                    if g % 2 == 0:
                        awt = awts[g // 2]
                        aoff = g * 4 * FI
                    if g == qstart:
                        ng = QUADS[qi]
                        stg = stgp.tile([BL, 4 * ng * FI], F32R, tag=f"stg{ng}")
                    so = (g - qstart) * 4 * FI
                    for u in range(4):
                        j = 4 * g + u
                        ps = apsum.tile([BL, FI], F32, tag="ap")
                        nc.tensor.matmul(
                            ps[:, :],
                            cT_sb[:, :],
                            awt[:, j * FI - aoff : (j + 1) * FI - aoff],
                            start=True,
                            stop=True,
                        )
                        if u % 2 == 0:
                            nc.scalar.activation(
                                stg[:, so + u * FI : so + (u + 1) * FI], ps[:, :], AF.Copy
                            )
                        else:
                            nc.vector.tensor_copy(
                                stg[:, so + u * FI : so + (u + 1) * FI], ps[:, :]
                            )
                    if g == QEND[qi]:
                        # the two samples' scatters go to different queues so
                        # their single-partition-line transfers run on two
                        # DMA engines concurrently instead of serializing
                        for b in range(BL):
                            eng = nc.sync if b == 0 else nc.gpsimd
                            eng.dma_start(
                                kern_raw[CH * b + 4 * qstart : CH * b + 4 * (g + 1), :],
                                stg[b : b + 1, :],
                            )
                        qstart = g + 1
                        qi += 1
                    if g == 6:
                        nc.sync.dma_start(ab_sb[:, :], ab_d[:, :])
                        nc.sync.dma_start(rw2_sb[:, :], rw2_d[:, :])
                        nc.sync.dma_start(rb2_sb[:, :], rb2_d[:, :])
                # x chunks queue behind the scatters on the sync queue, so
                # the bulk x flood can't delay the kernel assembly.
                XCHUNKS = [1024] + [2048] * 7 + [1024]
                xq = 0
                for xl in XCHUNKS:
                    nc.sync.dma_start(
                        xbuf[:, PAD + xq : PAD + xq + xl],
                        x_in[:, xq : xq + xl],
                    )
                    xq += xl
                # block-diagonal paired tiles built directly from kern_raw
                # + host-prelayouted adapter bias (ab2kt is already in KT
                # coordinates, zeros in the off-diagonal blocks); T blocks
                # first so the conv's first matmuls unblock earliest
                for k in range(K):
                    nc.vector.tensor_add(
                        kt[0:CH, 128 * k : 128 * k + CH],
                        kern_raw[0:CH, 128 * k : 128 * k + CH],
                        ab_sb[0:CH, 128 * k : 128 * k + CH],
                    )
                    nc.vector.tensor_add(
                        kt[CH:P, 128 * k + CH : 128 * (k + 1)],
                        kern_raw[CH:P, 128 * k : 128 * k + CH],
                        ab_sb[CH:P, 128 * k : 128 * k + CH],
                    )
                for k in range(K):
                    nc.vector.tensor_add(
                        kt[0:CH, FI + 128 * k : FI + 128 * k + CH],
                        kern_raw[0:CH, 128 * k + CH : 128 * (k + 1)],
                        ab_sb[0:CH, 128 * k + CH : 128 * (k + 1)],
                    )
                    nc.vector.tensor_add(
                        kt[CH:P, FI + 128 * k + CH : FI + 128 * (k + 1)],
                        kern_raw[CH:P, 128 * k + CH : 128 * (k + 1)],
                        ab_sb[CH:P, 128 * k + CH : 128 * (k + 1)],
                    )

            # ---------------- phase B: conv + gate + residual --------------
            # The residual matmul for block j-1 is emitted after block j's
            # conv matmuls: by then z2[j-1] is long finished, so the PE
            # stream never stalls on the gate.  (+rb +x) is one fused DVE op.
            with (
                tc.tile_pool(name="ppool", bufs=2, space="PSUM") as ppool,
                tc.tile_pool(name="spool", bufs=2, space="PSUM") as spool,
                tc.tile_pool(name="work", bufs=2) as workp,
                tc.tile_pool(name="store", bufs=4) as storep,
            ):
                z2s = [None] * NJ
                po2s = [None] * NJ

                def emit_residual(jr):
                    po2 = ppool.tile([P, UW], F32, tag="acc")
                    po2s[jr] = po2
                    for h in range(UW // NT):
                        nc.tensor.matmul(
                            po2[:, h * NT : (h + 1) * NT],
                            rw2_sb[:, :],
                            z2s[jr][:, h * NT : (h + 1) * NT],
                            start=True,
                            stop=True,
                        )

                def emit_post(jr):
                    c0r = jr * UW
                    ot = storep.tile([P, UW], F32, tag="ot")
                    nc.vector.scalar_tensor_tensor(
                        ot[:, :],
                        po2s[jr][:, :],
                        rb2_sb[:, 0:1],
                        xbuf[:, c0r + PAD : c0r + PAD + UW],
                        op0=mybir.AluOpType.add,
                        op1=mybir.AluOpType.add,
                    )
                    nc.gpsimd.dma_start(out_d[:, c0r : c0r + UW], ot[:, :])

                for j in range(NJ):
                    c0 = j * UW
                    py_t = ppool.tile([P, UW], F32, tag="acc")
                    py_s = spool.tile([P, UW], F32, tag="pys")
                    for k in range(K):
                        for h in range(UW // NT):
                            nc.tensor.matmul(
                                py_t[:, h * NT : (h + 1) * NT],
                                kt[:, 128 * k : 128 * (k + 1)],
                                xbuf[:, c0 + h * NT + DIL * k : c0 + h * NT + DIL * k + NT],
                                start=(k == 0),
                                stop=(k == K - 1),
                            )
                    for k in range(K):
                        for h in range(UW // NT):
                            nc.tensor.matmul(
                                py_s[:, h * NT : (h + 1) * NT],
                                kt[:, FI + 128 * k : FI + 128 * (k + 1)],
                                xbuf[:, c0 + h * NT + DIL * k : c0 + h * NT + DIL * k + NT],
                                start=(k == 0),
                                stop=(k == K - 1),
                            )
                    if j > 0:
                        emit_residual(j - 1)
                    ta = workp.tile([P, UW], F32R, tag="ta")
                    nc.scalar.activation(ta[:, :], py_t[:, :], AF.Tanh, bias=bt2[:, 0:1])
                    ts = workp.tile([P, UW], F32R, tag="ts")
                    nc.scalar.activation(ts[:, :], py_s[:, :], AF.Sigmoid, bias=bs2[:, 0:1])
                    z2 = storep.tile([P, UW], F32R, tag="z2")
                    z2s[j] = z2
                    nc.vector.tensor_mul(z2[:, :], ta[:, :], ts[:, :])
                    nc.gpsimd.dma_start(z_d[:, c0 : c0 + UW], z2[:, :])
                    if j > 0:
                        emit_post(j - 1)
                emit_residual(NJ - 1)
                emit_post(NJ - 1)

    nc.compile()
    return nc


def get_nc():
    global _NC
    if _NC is None:
        _NC = _build()
    return _NC


def make_in_maps(inputs):
    x = np.ascontiguousarray(np.asarray(inputs["x"], np.float32))
    c = np.asarray(inputs["c"], np.float32)
    aw = np.asarray(inputs["adapter_w"], np.float32)
    ab = np.asarray(inputs["adapter_b"], np.float32)
    bw = np.ascontiguousarray(np.asarray(inputs["bias_w"], np.float32))
    bb = np.asarray(inputs["bias_b"], np.float32).reshape(1, 2 * CH)
    rw = np.asarray(inputs["resi_w"], np.float32)
    rb = np.asarray(inputs["resi_b"], np.float32)

    # adapter columns [cond, (o,i,k)] -> [cond, (i,k,o)]
    aw_r = np.ascontiguousarray(
        aw.reshape(COND, 2 * CH, CH, K).transpose(0, 2, 3, 1).reshape(COND, F)
    )
    ab_r1 = ab.reshape(2 * CH, CH, K).transpose(1, 2, 0).reshape(CH, FI)
    ab_r = np.ascontiguousarray(np.concatenate([ab_r1, ab_r1], axis=0))
    # paired residual weights: blockdiag(rw.T, rw.T), [rb; rb]
    rw2 = np.zeros((P, P), np.float32)
    rw2[0:CH, 0:CH] = rw.T
    rw2[CH:P, CH:P] = rw.T
    rb2 = np.concatenate([rb, rb]).reshape(P, 1).astype(np.float32)

    import ml_dtypes

    adt = ml_dtypes.bfloat16 if ADAPTER_BF16 else np.float32
    aw_bf = np.ascontiguousarray(aw_r.astype(adt))
    in_maps = []
    for m in range(NCORES):
        sl = slice(BL * m, BL * (m + 1))
        in_maps.append(
            {
                "x_in": np.ascontiguousarray(x[sl].reshape(P, T)),
                "cT": np.ascontiguousarray(c[sl].T.astype(adt)),
                "cTf": np.ascontiguousarray(c[sl].T),
                "aw_r": aw_bf,
                "ab_r": ab_r,
                "bw": bw,
                "bb": bb,
                "rw2": rw2,
                "rb2": rb2,
            }
        )
    return in_maps


def kernel(**inputs):
    global LAST_RESULTS
    nc = get_nc()
    in_maps = make_in_maps(inputs)
    res = run_bass_kernel_spmd(
        nc, in_maps, list(range(NCORES)), trace=TRACE
    )
    LAST_RESULTS = res
    out = np.empty((B, CH, T), np.float32)
    z = np.empty((B, CH, T), np.float32)
    for m in range(NCORES):
        out[BL * m : BL * (m + 1)] = res.results[m]["out_d"].reshape(BL, CH, T)
        z[BL * m : BL * (m + 1)] = res.results[m]["z_d"].reshape(BL, CH, T)
    return out, z


# revision 43
# speedup vs baseline: 1.0824x; 1.0402x over previous
"""Gated TCN layer (fully conditioned) as a Bass/Tile kernel on 8 NeuronCores.

Reference computation (per sample b):
    kern = (c @ adapter_w + adapter_b).reshape(2*CH, CH, K)
    y    = dilated causal conv of x with per-sample kern (K=3, dil=4)
    y   += (c @ bias_w + bias_b)[:, None]
    z    = tanh(y[:CH]) * sigmoid(y[CH:])
    out  = resi_w @ z + resi_b + x
Returns (out, z).

Sharding: data-parallel over batch, 2 samples per core.  The two samples
are packed on the 128 SBUF partitions (sample 0 on 0-63, sample 1 on
64-127) so conv / activations / residual all run at full 128-partition
width.  Conv weights are block-diagonal per tap: T_k pairs the tanh
halves of both samples, S_k the sigmoid halves, giving 128-row PE
contraction per matmul.  The residual rw2 @ z2 runs software-pipelined
one block behind the conv so the PE never waits on the gate output;
(+ rb + x) is a single fused DVE op.
"""

import numpy as np

from concourse import bacc, mybir, tile
from concourse.bass_utils import run_bass_kernel_spmd

K = 3
DIL = 4
CH = 64
COND = 128
B, T = 16, 16384
NCORES = 8
BL = B // NCORES          # samples per core
PAD = (K - 1) * DIL       # causal left pad = 8
NT = 512                  # matmul free-dim (one PSUM bank of fp32)
UW = 1024                 # processing unit width (2 PSUM banks)
NJ = T // UW
F = K * CH * 2 * CH       # 24576 adapter columns
FI = 2 * CH * K           # 384 adapter columns per input-channel row
XCH = 2048                # x load chunk (columns)
P = BL * CH               # 128 partitions = both samples' channels

F32 = mybir.dt.float32
F32R = mybir.dt.float32r
BF16 = mybir.dt.bfloat16
AF = mybir.ActivationFunctionType

ADAPTER_BF16 = True

# Set by test.py to capture a profile; harness path leaves these alone.
TRACE = False
LAST_RESULTS = None

_NC = None


def _build():
    nc = bacc.Bacc("TRN2", target_bir_lowering=False, debug=False)

    x_in = nc.dram_tensor("x_in", [P, T], F32R, kind="ExternalInput")
    cT_d = nc.dram_tensor("cT", [COND, BL], BF16 if ADAPTER_BF16 else F32R, kind="ExternalInput")
    cTf_d = nc.dram_tensor("cTf", [COND, BL], F32R, kind="ExternalInput")
    aw_d = nc.dram_tensor("aw_r", [COND, F], BF16 if ADAPTER_BF16 else F32R, kind="ExternalInput")
    ab_d = nc.dram_tensor("ab_r", [P, FI], F32R, kind="ExternalInput")
    bw_d = nc.dram_tensor("bw", [COND, 2 * CH], F32R, kind="ExternalInput")
    bb_d = nc.dram_tensor("bb", [1, 2 * CH], F32R, kind="ExternalInput")
    rw2_d = nc.dram_tensor("rw2", [P, P], F32R, kind="ExternalInput")
    rb2_d = nc.dram_tensor("rb2", [P, 1], F32, kind="ExternalInput")
    out_d = nc.dram_tensor("out_d", [P, T], F32, kind="ExternalOutput")
    z_d = nc.dram_tensor("z_d", [P, T], F32R, kind="ExternalOutput")  # f32r == f32 bits

    with tile.TileContext(nc) as tc:
        with (
            tc.tile_pool(name="const", bufs=1) as constp,
            tc.tile_pool(name="xpool", bufs=1) as xpool,
            tc.tile_pool(name="kern", bufs=1) as kernp,
        ):
            # cT first (the only input the adapter matmuls need beyond aw);
            # every descriptor ahead of the aw chunks delays them by the
            # ~620ns dma_start issue cost, so nothing else goes first.
            cT_sb = constp.tile([COND, BL], BF16 if ADAPTER_BF16 else F32R)
            nc.sync.dma_start(cT_sb[:, :], cT_d[:, :])
            cTf_sb = constp.tile([COND, BL], F32R)
            bw_sb = constp.tile([COND, 2 * CH], F32R)
            bb_sb = constp.tile([1, 2 * CH], F32R)
            ones_sb = constp.tile([1, BL], F32R)
            nc.vector.memset(ones_sb[:, :].bitcast(F32), 1.0)

            xbuf = xpool.tile([P, PAD + T], F32R)
            nc.vector.memset(xbuf[:, 0:PAD].bitcast(F32), 0.0)

            ab_sb = constp.tile([P, FI], F32R)
            rw2_sb = constp.tile([P, P], F32R)
            rb2_sb = constp.tile([P, 1], F32)

            # Per-sample dynamic kernels: kfin[64b+i, 128k+o] = kern_b[o,i,k]+ab.
            # KT holds the block-diagonal paired weights:
            #   cols [128k,128k+128)      = T_k (tanh pair of tap k)
            #   cols [384+128k, ...)      = S_k (sigmoid pair of tap k)
            kern_raw = kernp.tile([P, FI], F32R, name="kern_raw")
            kt = kernp.tile([P, 2 * FI], F32R, name="kt")
            nc.vector.memset(kt[:, :].bitcast(F32), 0.0)
            bias_sb = constp.tile([2 * CH, BL], F32)
            bt2 = constp.tile([P, 1], F32)   # paired tanh bias
            bs2 = constp.tile([P, 1], F32)   # paired sigmoid bias

            # ---------------- phase A: adapter + conditioned bias ----------
            with (
                tc.tile_pool(name="awp", bufs=4) as awp,
                tc.tile_pool(name="apsum", bufs=7, space="PSUM") as apsum,
                tc.tile_pool(name="stg", bufs=3) as stgp,
                tc.tile_pool(name="bpsum", bufs=1, space="PSUM") as bpsum,
            ):
                # all adapter-weight chunks issued up front on the sync queue
                # (6KB per-partition lines: the DMA engines' sweet spot)
                awts = []
                for c in range(8):
                    awt = awp.tile([COND, 8 * FI], BF16 if ADAPTER_BF16 else F32R, tag="aw")
                    nc.sync.dma_start(
                        awt[:, :], aw_d[:, c * 8 * FI : (c + 1) * 8 * FI]
                    )
                    awts.append(awt)
                # bias inputs ride the idle gpsimd queue so the bias
                # matmul (and the paired-bias DMAs below) finish early
                nc.gpsimd.dma_start(cTf_sb[:, :], cTf_d[:, :])
                nc.gpsimd.dma_start(bw_sb[:, :], bw_d[:, :])
                nc.gpsimd.dma_start(bb_sb[:, :], bb_d[:, :])
                # first x chunk early on the gpsimd queue: small enough not
                # to disturb the aw stream, and it unblocks conv block 0
                nc.gpsimd.dma_start(
                    xbuf[:, PAD : PAD + 1024], x_in[:, 0:1024]
                )
                pb = bpsum.tile([2 * CH, BL], F32)
                nc.tensor.matmul(
                    pb[:, :], bw_sb[:, :], cTf_sb[:, :], start=True, stop=False
                )
                nc.tensor.matmul(
                    pb[:, :], bb_sb[:, :], ones_sb[:, :], start=False, stop=True
                )
                nc.vector.tensor_copy(bias_sb[:, :], pb[:, :])
                # paired per-partition bias tiles: issued now (the sync
                # engine only waits ~1us for the pb copy here), keeping these
                # 4 descriptors out of the critical post-adapter window
                for b in range(BL):
                    nc.sync.dma_start(
                        bt2[CH * b : CH * (b + 1), :], bias_sb[0:CH, b : b + 1]
                    )
                    nc.sync.dma_start(
                        bs2[CH * b : CH * (b + 1), :], bias_sb[CH : 2 * CH, b : b + 1]
                    )

                # 16 groups of 4 input-channel rows, staged in quads of
                # [4,4,4,2,2] groups: few scatters (dma_start issue costs
                # ~620ns), with small final quads so the last scatters'
                # single-partition-line transfer latency doesn't delay the
                # kernel assembly.
                QUADS = [4, 4, 4, 2, 2]
                QEND = []
                acc = 0
                for ng in QUADS:
                    acc += ng
                    QEND.append(acc - 1)
                qi = 0
                qstart = 0
                stg = None
                for g in range(CH // 4):
                    if g % 2 == 0:
                        awt = awts[g // 2]
                        aoff = g * 4 * FI
                    if g == qstart:
                        ng = QUADS[qi]
                        stg = stgp.tile([BL, 4 * ng * FI], F32R, tag=f"stg{ng}")
                    so = (g - qstart) * 4 * FI
                    for u in range(4):
                        j = 4 * g + u
                        ps = apsum.tile([BL, FI], F32, tag="ap")
                        nc.tensor.matmul(
                            ps[:, :],
                            cT_sb[:, :],
                            awt[:, j * FI - aoff : (j + 1) * FI - aoff],
                            start=True,
                            stop=True,
                        )
                        if u % 2 == 0:
                            nc.scalar.activation(
                                stg[:, so + u * FI : so + (u + 1) * FI], ps[:, :], AF.Copy
                            )
                        else:
                            nc.vector.tensor_copy(
                                stg[:, so + u * FI : so + (u + 1) * FI], ps[:, :]
                            )
                    if g == QEND[qi]:
                        # the two samples' scatters go to different queues so
                        # their single-partition-line transfers run on two
                        # DMA engines concurrently instead of serializing
                        for b in range(BL):
                            eng = nc.sync if b == 0 else nc.gpsimd
                            eng.dma_start(
                                kern_raw[CH * b + 4 * qstart : CH * b + 4 * (g + 1), :],
                                stg[b : b + 1, :],
                            )
                        qstart = g + 1
                        qi += 1
                    if g == 6:
                        nc.sync.dma_start(ab_sb[:, :], ab_d[:, :])
                        nc.sync.dma_start(rw2_sb[:, :], rw2_d[:, :])
                        nc.sync.dma_start(rb2_sb[:, :], rb2_d[:, :])
                # x chunks queue behind the scatters on the sync queue, so
                # the bulk x flood can't delay the kernel assembly.
                XCHUNKS = [2048] * 7 + [1024]
                xq = 1024
                for xl in XCHUNKS:
                    nc.sync.dma_start(
                        xbuf[:, PAD + xq : PAD + xq + xl],
                        x_in[:, xq : xq + xl],
                    )
                    xq += xl
                # block-diagonal paired tiles built directly from kern_raw
                # + host-prelayouted adapter bias (ab2kt is already in KT
                # coordinates, zeros in the off-diagonal blocks); T blocks
                # first so the conv's first matmuls unblock earliest
                for k in range(K):
                    nc.vector.tensor_add(
                        kt[0:CH, 128 * k : 128 * k + CH],
                        kern_raw[0:CH, 128 * k : 128 * k + CH],
                        ab_sb[0:CH, 128 * k : 128 * k + CH],
                    )
                    nc.vector.tensor_add(
                        kt[CH:P, 128 * k + CH : 128 * (k + 1)],
                        kern_raw[CH:P, 128 * k : 128 * k + CH],
                        ab_sb[CH:P, 128 * k : 128 * k + CH],
                    )
                for k in range(K):
                    nc.vector.tensor_add(
                        kt[0:CH, FI + 128 * k : FI + 128 * k + CH],
                        kern_raw[0:CH, 128 * k + CH : 128 * (k + 1)],
                        ab_sb[0:CH, 128 * k + CH : 128 * (k + 1)],
                    )
                    nc.vector.tensor_add(
                        kt[CH:P, FI + 128 * k + CH : FI + 128 * (k + 1)],
                        kern_raw[CH:P, 128 * k + CH : 128 * (k + 1)],
                        ab_sb[CH:P, 128 * k + CH : 128 * (k + 1)],
                    )

            # ---------------- phase B: conv + gate + residual --------------
            # The residual matmul for block j-1 is emitted after block j's
            # conv matmuls: by then z2[j-1] is long finished, so the PE
            # stream never stalls on the gate.  (+rb +x) is one fused DVE op.
            with (
                tc.tile_pool(name="ppool", bufs=2, space="PSUM") as ppool,
                tc.tile_pool(name="spool", bufs=2, space="PSUM") as spool,
                tc.tile_pool(name="work", bufs=2) as workp,
                tc.tile_pool(name="store", bufs=4) as storep,
            ):
                z2s = [None] * NJ
                po2s = [None] * NJ

                def emit_residual(jr):
                    po2 = ppool.tile([P, UW], F32, tag="acc")
                    po2s[jr] = po2
                    for h in range(UW // NT):
                        nc.tensor.matmul(
                            po2[:, h * NT : (h + 1) * NT],
                            rw2_sb[:, :],
                            z2s[jr][:, h * NT : (h + 1) * NT],
                            start=True,
                            stop=True,
                        )

                def emit_post(jr):
                    c0r = jr * UW
                    ot = storep.tile([P, UW], F32, tag="ot")
                    nc.vector.scalar_tensor_tensor(
                        ot[:, :],
                        po2s[jr][:, :],
                        rb2_sb[:, 0:1],
                        xbuf[:, c0r + PAD : c0r + PAD + UW],
                        op0=mybir.AluOpType.add,
                        op1=mybir.AluOpType.add,
                    )
                    nc.gpsimd.dma_start(out_d[:, c0r : c0r + UW], ot[:, :])

                for j in range(NJ):
                    c0 = j * UW
                    py_t = ppool.tile([P, UW], F32, tag="acc")
                    py_s = spool.tile([P, UW], F32, tag="pys")
                    for k in range(K):
                        for h in range(UW // NT):
                            nc.tensor.matmul(
                                py_t[:, h * NT : (h + 1) * NT],
                                kt[:, 128 * k : 128 * (k + 1)],
                                xbuf[:, c0 + h * NT + DIL * k : c0 + h * NT + DIL * k + NT],
                                start=(k == 0),
                                stop=(k == K - 1),
                            )
                    for k in range(K):
                        for h in range(UW // NT):
                            nc.tensor.matmul(
                                py_s[:, h * NT : (h + 1) * NT],
                                kt[:, FI + 128 * k : FI + 128 * (k + 1)],
                                xbuf[:, c0 + h * NT + DIL * k : c0 + h * NT + DIL * k + NT],
                                start=(k == 0),
                                stop=(k == K - 1),
                            )
                    if j > 0:
                        emit_residual(j - 1)
                    ta = workp.tile([P, UW], F32R, tag="ta")
                    nc.scalar.activation(ta[:, :], py_t[:, :], AF.Tanh, bias=bt2[:, 0:1])
                    ts = workp.tile([P, UW], F32R, tag="ts")
                    nc.scalar.activation(ts[:, :], py_s[:, :], AF.Sigmoid, bias=bs2[:, 0:1])
                    z2 = storep.tile([P, UW], F32R, tag="z2")
                    z2s[j] = z2
                    nc.vector.tensor_mul(z2[:, :], ta[:, :], ts[:, :])
                    nc.gpsimd.dma_start(z_d[:, c0 : c0 + UW], z2[:, :])
                    if j > 0:
                        emit_post(j - 1)
                # drain the last block in 512-col halves, stores on both
                # queues, so the tail chain pipelines instead of serializing
                jr = NJ - 1
                c0r = jr * UW
                po2 = ppool.tile([P, UW], F32, tag="acc")
                ot = storep.tile([P, UW], F32, tag="ot")
                for h in range(UW // NT):
                    hs = slice(h * NT, (h + 1) * NT)
                    nc.tensor.matmul(
                        po2[:, hs],
                        rw2_sb[:, :],
                        z2s[jr][:, hs],
                        start=True,
                        stop=True,
                    )
                    nc.vector.scalar_tensor_tensor(
                        ot[:, hs],
                        po2[:, hs],
                        rb2_sb[:, 0:1],
                        xbuf[:, c0r + PAD + h * NT : c0r + PAD + (h + 1) * NT],
                        op0=mybir.AluOpType.add,
                        op1=mybir.AluOpType.add,
                    )
                    eng = nc.sync if h == 0 else nc.gpsimd
                    eng.dma_start(
                        out_d[:, c0r + h * NT : c0r + (h + 1) * NT], ot[:, hs]
                    )

    nc.compile()
    return nc


def get_nc():
    global _NC
    if _NC is None:
        _NC = _build()
    return _NC


def make_in_maps(inputs):
    x = np.ascontiguousarray(np.asarray(inputs["x"], np.float32))
    c = np.asarray(inputs["c"], np.float32)
    aw = np.asarray(inputs["adapter_w"], np.float32)
    ab = np.asarray(inputs["adapter_b"], np.float32)
    bw = np.ascontiguousarray(np.asarray(inputs["bias_w"], np.float32))
    bb = np.asarray(inputs["bias_b"], np.float32).reshape(1, 2 * CH)
    rw = np.asarray(inputs["resi_w"], np.float32)
    rb = np.asarray(inputs["resi_b"], np.float32)

    # adapter columns [cond, (o,i,k)] -> [cond, (i,k,o)]
    aw_r = np.ascontiguousarray(
        aw.reshape(COND, 2 * CH, CH, K).transpose(0, 2, 3, 1).reshape(COND, F)
    )
    ab_r1 = ab.reshape(2 * CH, CH, K).transpose(1, 2, 0).reshape(CH, FI)
    ab_r = np.ascontiguousarray(np.concatenate([ab_r1, ab_r1], axis=0))
    # paired residual weights: blockdiag(rw.T, rw.T), [rb; rb]
    rw2 = np.zeros((P, P), np.float32)
    rw2[0:CH, 0:CH] = rw.T
    rw2[CH:P, CH:P] = rw.T
    rb2 = np.concatenate([rb, rb]).reshape(P, 1).astype(np.float32)

    import ml_dtypes

    adt = ml_dtypes.bfloat16 if ADAPTER_BF16 else np.float32
    aw_bf = np.ascontiguousarray(aw_r.astype(adt))
    in_maps = []
    for m in range(NCORES):
        sl = slice(BL * m, BL * (m + 1))
        in_maps.append(
            {
                "x_in": np.ascontiguousarray(x[sl].reshape(P, T)),
                "cT": np.ascontiguousarray(c[sl].T.astype(adt)),
                "cTf": np.ascontiguousarray(c[sl].T),
                "aw_r": aw_bf,
                "ab_r": ab_r,
                "bw": bw,
                "bb": bb,
                "rw2": rw2,
                "rb2": rb2,
            }
        )
    return in_maps


def kernel(**inputs):
    global LAST_RESULTS
    nc = get_nc()
    in_maps = make_in_maps(inputs)
    res = run_bass_kernel_spmd(
        nc, in_maps, list(range(NCORES)), trace=TRACE
    )
    LAST_RESULTS = res
    out = np.empty((B, CH, T), np.float32)
    z = np.empty((B, CH, T), np.float32)
    for m in range(NCORES):
        out[BL * m : BL * (m + 1)] = res.results[m]["out_d"].reshape(BL, CH, T)
        z[BL * m : BL * (m + 1)] = res.results[m]["z_d"].reshape(BL, CH, T)
    return out, z
